# revision 13
# baseline (speedup 1.0000x reference)
"""Cosine-similarity attention kernel for Trainium2 (8 NeuronCores, SPMD).

Problem: B=4, D=1024, T=2048, n_head=8, alpha=5.0.
Math (per batch b, head h, with d = D/8 = 128):
    qn = l2norm(q, axis=d); kn = l2norm(k, axis=d)
    S  = alpha * qn^T kn          [Tq, Tk]
    P  = softmax(S, axis=Tk)
    out= v @ P^T                  [dv, Tq]

Sharding: head-parallel — the 32 (b, h) pairs are split 4-per-core across
8 cores. Each core computes full attention for its 4 pairs.

Design notes (v3; v2 was PE-bound at 175us busy with ACT at 165):
  - All PE matmuls in bf16; q/k/v converted to bf16 on the HOST.
  - Scores computed transposed (S^T = kn^T @ qn, [k, q] layout) so the AV
    matmul contracts over k on the partition dim. No softmax max-subtraction:
    |S| <= alpha = 5, exp in [e-5, e5] is fp32/bf16-safe.
  - Softmax denominators: instead of 16 all-ones rowsum matmuls per qb
    (1/3 of all PE work in v2), exp chunks are folded pairwise first:
    level-1 folds (e[:, :512]+e[:, 512:] per chunk) run on the otherwise
    idle GPSIMD/Pool engine (6 of 8) and DVE (2 of 8); level-2 folds
    (DVE, bf16 2x) reduce 8 -> 4 tiles; only 4 rowsum matmuls remain.
    bf16 fold rounding adds ~0.05% to the denominator - well under the
    bf16 noise already present in e itself.
  - Norms: the v2 kernel burned ~60us of DVE on a fast-inv-sqrt chain over
    [128, T] broadcast data (128x redundant) plus ~20us of ACT on Ln/Exp.
    v3 computes the rsqrt chain on COMPACT data: ssq ones-matmul (PE,
    broadcast by construction) -> DVE stream-transpose of a 32-partition
    slice -> strided extract to [32, 64] -> 8-op magic+Halley chain on
    [32, 128] (q and k jointly, ~2us/pair) -> PE transpose to [128, 32]
    -> DMA row-build -> 7 doubling DMAs replicate to a [128, T] bf16
    broadcast tile -> one bf16 2x multiply materializes qn/kn.
    Pair 0 keeps the v2 ACT path (Ln+Exp) for both tensors: ACT is idle
    during the prologue and the compact pipeline is longer end-to-end.
  - PSUM: 3 score-chunk bufs [128,1024] (6 banks) + av + rs (1 bank each).
    Norm machinery borrows chunk bufs (ssq) and the rs bank (transpose).
  - One-chunk software pipeline as in v2: av matmuls/folds for chunk c are
    emitted after the scores matmuls of chunk c+1; drains (av copy,
    reciprocal_approx_fast, multiply - all DVE) land inside the next qb.
"""

import math
import os
import sys
from contextlib import ExitStack

for _p in ("/opt/trn_rl_repo", "/root/.axon_site/_ro/trn_rl_repo"):
    if os.path.isdir(_p) and _p not in sys.path:
        sys.path.insert(0, _p)

import numpy as np
import ml_dtypes

import concourse.bass as bass
import concourse.tile as tile
from concourse import bacc, mybir
from concourse.bass_utils import run_bass_kernel_spmd
from concourse.masks import make_identity

N_CORES = 8
B, DFULL, T = 4, 1024, 2048
NHEAD = 8
D = DFULL // NHEAD          # 128 per-head channels
PAIRS = (B * NHEAD) // N_CORES  # 4 (b, h) pairs per core
ALPHA = 5.0

NKT = T // 128              # 16 k-tiles of 128
QB = 512                    # q-block width
NQB = T // SQB if False else T // 512  # 4 q-blocks
CHUNKS = [3, 3, 3, 3, 3, 1]  # k-tiles per exp chunk (2 x 3-bank psum bufs)
NORM_QB = 1024              # pair-0 norm processing chunk width
POOL_L1 = 7                 # level-1 folds per qb on GPSIMD (rest on DVE)

F32 = mybir.dt.float32
BF16 = mybir.dt.bfloat16
I32 = mybir.dt.int32
EXP = mybir.ActivationFunctionType.Exp
LN = mybir.ActivationFunctionType.Ln

RSQRT_MAGIC = 0x5F3759DF


class _PinnedActBacc(bacc.Bacc):
    """Bacc whose activation-table chooser is pinned so Exp and Ln both
    resolve to natural_log_exp_and_others (avoids per-alternation table
    loads)."""

    def insert_act_table_loads(self):
        import bass_rust as _bass_rust
        from concourse.hw_specs import get_activation_tables

        has_activation = any(
            isinstance(i, mybir.InstActivation)
            for b in self.main_func.blocks
            for i in b.instructions
        )
        if not has_activation:
            return
        keep = "natural_log_exp_and_others"
        drop = {
            mybir.ActivationFunctionType.Exp,
            mybir.ActivationFunctionType.Ln,
        }
        tables = []
        for name, fns in get_activation_tables(self.m.arch).items():
            tables.append((name, fns if name == keep else (fns - drop)))
        _bass_rust.insert_act_table_loads(self, tables)


def _build_nc(repeat: int = 1) -> bass.Bass:
    nc = _PinnedActBacc(None, target_bir_lowering=False)
    q_d = nc.declare_dram_parameter("q", [PAIRS, D, T], BF16, isOutput=False)
    k_d = nc.declare_dram_parameter("k", [PAIRS, D, T], BF16, isOutput=False)
    vt_d = nc.declare_dram_parameter("vt", [PAIRS, T, D], BF16, isOutput=False)
    out_d = nc.declare_dram_parameter("out", [PAIRS, D, T], F32, isOutput=True)

    with ExitStack() as ctx:
        tc = ctx.enter_context(tile.TileContext(nc))
        const_p = ctx.enter_context(tc.tile_pool(name="const", bufs=1))
        io_p = ctx.enter_context(tc.tile_pool(name="io", bufs=2))
        work_p = ctx.enter_context(tc.tile_pool(name="work", bufs=2))
        e_p = ctx.enter_context(tc.tile_pool(name="e", bufs=12))
        f_p = ctx.enter_context(tc.tile_pool(name="f", bufs=10))
        out_p = ctx.enter_context(tc.tile_pool(name="outp", bufs=3))
        # PSUM: chunk pool 3x[128,1024] (6 banks) + av 1x[128,512] (1 bank)
        # + rs 1x[128,512] (1 bank) = 8 banks.
        cps = ctx.enter_context(tc.tile_pool(name="cps", bufs=3, space="PSUM"))
        avps = ctx.enter_context(tc.tile_pool(name="avps", bufs=1, space="PSUM"))
        rsps = ctx.enter_context(tc.tile_pool(name="rsps", bufs=1, space="PSUM"))

        ones_f32 = const_p.tile([128, 128], F32)
        nc.vector.memset(ones_f32, 1.0)
        ones = const_p.tile([128, 128], BF16)
        nc.vector.tensor_copy(ones, ones_f32)
        ident = const_p.tile([32, 32], BF16)
        make_identity(nc, ident)
        # per-partition bias tile holding 0.5*ln(alpha): inv = sqrt(alpha)/||x||
        bias_hla = const_p.tile([128, 1], F32)
        nc.vector.memset(bias_hla, 0.5 * math.log(ALPHA))

        def emit_load(p):
            q_sb = io_p.tile([D, T], BF16, tag="q")
            k_sb = io_p.tile([D, T], BF16, tag="k")
            vt_sb = io_p.tile([128, NKT, D], BF16, tag="vt")
            for hh in range(2):
                sl = slice(hh * NORM_QB, (hh + 1) * NORM_QB)
                nc.sync.dma_start(out=q_sb[:, sl], in_=q_d[p][:, sl])
                nc.sync.dma_start(out=k_sb[:, sl], in_=k_d[p][:, sl])
            # vt dram [T, D] -> sbuf [128, kt, dv]: partition = k % 128
            nc.sync.dma_start(
                out=vt_sb,
                in_=vt_d[p].rearrange("(t kp) dv -> kp t dv", kp=128),
            )
            return q_sb, k_sb, vt_sb

        def norm_steps_act(x_sb, out):
            """Pair-0 path: exact inv via ACT (Ln then Exp) on the broadcast
            ssq. ACT is idle during the prologue, and this is much shorter
            end-to-end than the compact pipeline. One instruction per step."""
            for hh in range(2):
                xsl = x_sb[:, hh * NORM_QB:(hh + 1) * NORM_QB]
                sq = work_p.tile([D, NORM_QB], BF16, tag="sq")
                nc.vector.tensor_mul(sq, xsl, xsl)
                yield
                ssq = cps.tile([128, NORM_QB], F32, tag="chunk")
                for j in range(2):
                    nc.tensor.matmul(
                        ssq[:, j * 512:(j + 1) * 512],
                        lhsT=ones,
                        rhs=sq[:, j * 512:(j + 1) * 512],
                        start=True, stop=True,
                    )
                ssq_sb = work_p.tile([128, NORM_QB], F32, tag="ssq_sb")
                nc.vector.tensor_copy(ssq_sb, ssq)
                yield
                lnt = work_p.tile([128, NORM_QB], F32, tag="lnt")
                nc.scalar.activation(lnt, ssq_sb, LN)
                yield
                inv = work_p.tile([128, NORM_QB], F32, tag="invk")
                nc.scalar.activation(inv, lnt, EXP, scale=-0.5, bias=bias_hla)
                yield
                sl = slice(hh * NORM_QB, (hh + 1) * NORM_QB)
                nc.vector.tensor_mul(out[:, sl], x_sb[:, sl], inv)
                yield

        def norm_steps_compact(q_sb, k_sb, qn, kn):
            """Pairs 1-3: rsqrt chain on compact [32, 128] data (q cols 0-63,
            k cols 64-127), then PE-transpose + DMA row-build + 7 doubling
            DMAs to materialize a [128, T] bf16 broadcast inv tile per
            tensor. One instruction per step; driven from the main loop."""
            c = math.sqrt(ALPHA)
            cm = work_p.tile([32, 128], F32, tag="cm")
            st_tiles = []
            for ti, x_sb in enumerate((q_sb, k_sb)):
                sq = work_p.tile([D, T], BF16, tag="sq")
                nc.vector.tensor_mul(sq, x_sb, x_sb)
                yield
                for hh in range(2):
                    ssq = cps.tile([128, NORM_QB], F32, tag="chunk")
                    for j in range(2):
                        col = hh * NORM_QB + j * 512
                        nc.tensor.matmul(
                            ssq[:, j * 512:(j + 1) * 512],
                            lhsT=ones,
                            rhs=sq[:, col:col + 512],
                            start=True, stop=True,
                        )
                    # stream-transpose a 32-partition slice of the broadcast,
                    # reading it through a perfect-shuffle view (swap the two
                    # 5-bit halves of the column index): block B of the output
                    # then holds ssq(hh*1024 + 32i + B) at partition i,
                    # replicated along the block's 32 columns. The strided
                    # extract of col 0 per block gives a compact [32, 32]
                    # with t-high on partitions and t-low contiguous -> the
                    # later row-build DMA is 32 contiguous 64B descriptors.
                    st = work_p.tile([32, NORM_QB], F32, tag="st")
                    nc.vector.transpose(
                        st, ssq[0:32, :].rearrange("p (a b) -> p b a", a=32)
                    )
                    st_tiles.append((ti, hh, st))
                    yield
            for ti, hh, st in st_tiles:
                # cm[i, ti*64 + hh*32 + B] = ssq(t = hh*1024 + 32*i + B)
                dst = cm[:, ti * 64 + hh * 32: ti * 64 + (hh + 1) * 32]
                nc.vector.tensor_copy(
                    dst, st.rearrange("p (nb b) -> p nb b", b=32)[:, :, 0]
                )
            yield
            # fast inverse sqrt: magic + one Halley step (~1e-4 rel err),
            # with sqrt(alpha) folded into the constants.
            sh = work_p.tile([32, 128], I32, tag="csh")
            nc.vector.tensor_scalar(
                sh, cm.bitcast(I32), 1, None, mybir.AluOpType.arith_shift_right
            )
            yield
            nt = work_p.tile([32, 128], I32, tag="cnt")
            nc.vector.tensor_scalar(nt, sh, -1, None, mybir.AluOpType.bitwise_xor)
            yield
            y0i = work_p.tile([32, 128], I32, tag="cy0")
            nc.vector.tensor_scalar(
                y0i, nt, RSQRT_MAGIC + 1, None, mybir.AluOpType.add
            )
            yield
            y0 = y0i.bitcast(F32)
            a = work_p.tile([32, 128], F32, tag="ca")
            nc.vector.tensor_mul(a, cm, y0)
            yield
            h = work_p.tile([32, 128], F32, tag="chh")
            nc.vector.tensor_mul(h, a, y0)
            yield
            u = work_p.tile([32, 128], F32, tag="cu")
            nc.vector.tensor_scalar(
                u, h, 3.0 * c / 8.0, -10.0 * c / 8.0,
                mybir.AluOpType.mult, mybir.AluOpType.add,
            )
            yield
            p_t = work_p.tile([32, 128], F32, tag="cp")
            nc.vector.tensor_mul(p_t, h, u)
            yield
            cinv = work_p.tile([32, 128], BF16, tag="cinv")
            nc.vector.scalar_tensor_tensor(
                out=cinv, in0=p_t, scalar=15.0 * c / 8.0, in1=y0,
                op0=mybir.AluOpType.add, op1=mybir.AluOpType.mult,
            )
            yield
            invs = []
            for ti in range(2):
                # row-build straight from the compact sbuf tile: no PE
                # transpose (would contend for a PSUM bank mid-pair). Both
                # sides iterate (i, hh, b) so the byte sequences match:
                #   inv_row[0, t] = cinv[i, ti*64 + hh*32 + b],
                #   t = hh*1024 + 32*b + i
                # 2B elements at 64B stride -> 2048 descriptors, ~0.9us.
                inv_bc = work_p.tile([128, T], BF16, tag=f"invbc{ti}")
                for hh in range(2):
                    # inv_row[0, hh*1024 + 32p + f] = cinv[p, .. + f]:
                    # contiguous 64B runs on both sides
                    nc.sync.dma_start(
                        out=inv_bc[
                            0:1, hh * NORM_QB:(hh + 1) * NORM_QB
                        ].rearrange("one (p f) -> one p f", f=32),
                        in_=cinv[:, ti * 64 + hh * 32: ti * 64 + (hh + 1) * 32],
                    )
                invs.append(inv_bc)
                yield
            # one stride-0 broadcast DMA per tensor replicates partition 0
            # to partitions 1-127 (127 contiguous 4KB descriptors)
            for inv_bc in invs:
                nc.sync.dma_start(
                    out=inv_bc[1:128, :],
                    in_=inv_bc[0:1, :].unsqueeze(1).broadcast_to((1, 127, T)),
                )
                yield
            nc.vector.tensor_mul(qn, q_sb, invs[0])
            yield
            nc.vector.tensor_mul(kn, k_sb, invs[1])
            yield

        # software pipeline across pairs: pair p+1's loads and norms are
        # emitted between pair p's q-blocks to fill scheduler bubbles
        total = PAIRS * repeat
        cur_load = emit_load(0)

        qn0 = work_p.tile([D, T], BF16, tag="qn")
        kn0 = work_p.tile([D, T], BF16, tag="kn")
        pro = [norm_steps_act(cur_load[0], qn0),
               norm_steps_act(cur_load[1], kn0)]
        while pro:
            g = pro.pop(0)
            try:
                next(g)
                pro.append(g)
            except StopIteration:
                pass
        cur_norm = (qn0, kn0)
        nxt_load = None
        nxt_qn = nxt_kn = None
        gens = []
        # one-chunk software pipeline carried ACROSS qb and pair boundaries:
        # av matmuls + folds for chunk c are emitted after the scores matmuls
        # of the next chunk, so a marginally-late exp never head-of-line
        # blocks the in-order PE queue; each qb's drain lands inside the next
        # qb's stream.
        pending = []

        def emit_tail(pd):
            e_c, c, av_, rs_, vt_, l1s, drain_fn = pd
            for j in range(CK):
                kt = CK * c + j
                nc.tensor.matmul(
                    av_, lhsT=vt_[:, kt, :], rhs=e_c[:, j * 512:(j + 1) * 512],
                    start=(kt == 0), stop=(kt == NKT - 1),
                )
            # level-1 fold of this chunk's two e tiles (Pool for most,
            # DVE for the rest - Pool's software Add runs at 0.42 eff)
            l1 = f_p.tile([128, 512], BF16, tag="l1")
            eng = nc.gpsimd if (c % NCH) < POOL_L1 else nc.vector
            eng.tensor_add(l1, e_c[:, 0:512], e_c[:, 512:1024])
            l1s.append(l1)
            if (c % NCH) % 2 == 1:
                # level-2 fold (DVE) + one rowsum matmul on the folded tile
                i = (c % NCH) // 2
                l2 = f_p.tile([128, 512], BF16, tag="l2")
                nc.vector.tensor_add(l2, l1s[-2], l1s[-1])
                nc.tensor.matmul(
                    rs_, lhsT=ones, rhs=l2,
                    start=(i == 0), stop=(i == NCH // 2 - 1),
                )
            if drain_fn is not None:
                drain_fn()

        for p_rep in range(total):
            p = p_rep % PAIRS
            qn, kn = cur_norm
            vt_sb = cur_load[2]
            if p_rep + 1 < total:
                nxt_load = emit_load((p_rep + 1) % PAIRS)
                nxt_qn = work_p.tile([D, T], BF16, tag="qn")
                nxt_kn = work_p.tile([D, T], BF16, tag="kn")
                gens = [norm_steps_compact(nxt_load[0], nxt_load[1],
                                           nxt_qn, nxt_kn)]
            for qb in range(NQB):
                qsl = slice(qb * QB, (qb + 1) * QB)
                av = avps.tile([128, QB], F32, tag="av")
                rs = rsps.tile([128, QB], F32, tag="rs")
                l1s = []

                def mk_drain(av_, rs_, p_, qsl_):
                    def drain():
                        # DVE copy frees the single-buffered av bank in one
                        # ~700ns op; reciprocal+multiply then run from sbuf
                        av_sb = out_p.tile([128, QB], F32, tag="avsb")
                        nc.vector.tensor_copy(av_sb, av_)
                        invr = out_p.tile([128, QB], F32, tag="invr")
                        nc.vector.reciprocal_approx_fast(out=invr, in_=rs_)
                        o_sb = out_p.tile([128, QB], F32, tag="o")
                        nc.vector.tensor_mul(o_sb, av_sb, invr)
                        nc.sync.dma_start(out=out_d[p_][:, qsl_], in_=o_sb)
                    return drain

                for c in range(NCH):
                    sp = cps.tile([128, CK * 512], F32, tag="chunk")
                    for j in range(CK):
                        kt = CK * c + j
                        nc.tensor.matmul(
                            sp[:, j * 512:(j + 1) * 512],
                            lhsT=kn[:, kt * 128:(kt + 1) * 128],
                            rhs=qn[:, qsl],
                            start=True, stop=True,
                        )
                    e_c = e_p.tile([128, CK * 512], BF16, tag="e")
                    nc.scalar.activation(e_c, sp, EXP)
                    if len(pending) == 3:
                        emit_tail(pending.pop(0))
                    drain_fn = mk_drain(av, rs, p, qsl) if c == NCH - 1 else None
                    pending.append((e_c, c, av, rs, vt_sb, l1s, drain_fn))
                    # drive the next pair's norm chain so its work trickles
                    # into the queues
                    for _ in range(3):
                        if gens:
                            g = gens.pop(0)
                            try:
                                next(g)
                                gens.append(g)
                            except StopIteration:
                                pass
            for g in gens:
                for _ in g:
                    pass
            gens = []
            if p_rep + 1 < total:
                cur_load = nxt_load
                cur_norm = (nxt_qn, nxt_kn)
        for pd in pending:
            emit_tail(pd)
        pending = []

    nc.finalize()
    return nc


_NC_CACHE = None


def _get_nc() -> bass.Bass:
    global _NC_CACHE
    if _NC_CACHE is None:
        _NC_CACHE = _build_nc()
    return _NC_CACHE


def make_in_maps(q: np.ndarray, k: np.ndarray, v: np.ndarray):
    """Shard full [B, D, T] inputs into per-core in_maps (host-side bf16)."""
    bf = ml_dtypes.bfloat16
    qr = q.reshape(B * NHEAD, D, T).astype(bf)
    kr = k.reshape(B * NHEAD, D, T).astype(bf)
    vr = v.reshape(B * NHEAD, D, T).transpose(0, 2, 1).astype(bf)  # [32, T, d]
    in_maps = []
    for c in range(N_CORES):
        sl = slice(c * PAIRS, (c + 1) * PAIRS)
        in_maps.append({
            "q": np.ascontiguousarray(qr[sl]),
            "k": np.ascontiguousarray(kr[sl]),
            "vt": np.ascontiguousarray(vr[sl]),
        })
    return in_maps


def gather_out(results) -> np.ndarray:
    outs = np.concatenate(
        [results[c]["out"] for c in range(N_CORES)], axis=0
    )  # [32, d, T]
    return np.ascontiguousarray(outs.reshape(B, DFULL, T), dtype=np.float32)


def run(q, k, v, **kwargs):
    nc = _get_nc()
    res = run_bass_kernel_spmd(nc, make_in_maps(q, k, v), list(range(N_CORES)), **kwargs)
    return gather_out(res.results), res


def kernel(q: np.ndarray, k: np.ndarray, v: np.ndarray) -> np.ndarray:
    out, _ = run(q, k, v)
    return out
